# revision 1
# baseline (speedup 1.0000x reference)
"""
Trainium2 Bass kernel for AttnBlock++ (GroupNorm -> q/k/v NIN -> HWxHW
attention -> out NIN -> residual).

Sharding: 8 cores = 4 batches x 2 query-halves. Each core gets the full
[C, H*W] image of its batch (for GroupNorm stats, k and v) plus its query
half, and produces out[:, n_slice] for that half. No collectives.

Per-core kernel highlights:
  - GroupNorm is folded into the q/k/v weights (W' = s_c * W,
    b' = b + W^T t) so the normalized activations are never materialized.
  - All large matmuls run in float32r (~fp22) at full PE rate.
  - Attention pass 1 computes wT = k^T q directly (so no transpose of the
    attention matrix is ever needed); pass 2 uses exp(wT) tiles as lhsT
    against v^T augmented with a ones column, whose output column 256 is
    the softmax denominator.
"""

import sys

for _p in ("/opt/trn_rl_repo",):
    if _p not in sys.path:
        sys.path.insert(0, _p)

import numpy as np

B, C, H, W = 4, 256, 64, 64
N = H * W            # 4096 spatial positions
NCORES = 8
SPLIT = NCORES // B  # query-halves per batch
NQ = N // SPLIT      # 2048 query positions per core
P = 128              # SBUF partitions
CB = C // P          # channel blocks
G = 32               # groupnorm groups
GPB = P // (C // G)  # groups per channel block = 16
EPS = 1e-6
NT = 256             # attention n-tile width
MT = 512             # qkv m-tile width
XCH = 512            # x DMA chunk width
SCALE = float(C) ** -0.5

_prog = None


def _build_program():
    from concourse import bacc
    import concourse.mybir as mybir
    import concourse.tile as tile

    dt = mybir.dt
    f32 = dt.float32
    f32r = dt.float32r
    Act = mybir.ActivationFunctionType
    Alu = mybir.AluOpType

    nc = bacc.Bacc()

    xf = nc.dram_tensor("xf", [C, N], f32, kind="ExternalInput")
    xs = nc.dram_tensor("xs", [C, NQ], f32, kind="ExternalInput")
    Wd = {}
    bd = {}
    for nm in ("q", "k", "v", "o"):
        Wd[nm] = nc.dram_tensor(f"W{nm}", [C, C], f32, kind="ExternalInput")
        bd[nm] = nc.dram_tensor(f"b{nm}", [C], f32, kind="ExternalInput")
    gamma_d = nc.dram_tensor("gamma", [C], f32, kind="ExternalInput")
    beta_d = nc.dram_tensor("beta", [C], f32, kind="ExternalInput")
    ident_d = nc.dram_tensor("ident", [P, P], f32, kind="ExternalInput")
    sel8_d = nc.dram_tensor("sel8", [P, GPB], f32, kind="ExternalInput")
    sel8T_d = nc.dram_tensor("sel8T", [GPB, P], f32, kind="ExternalInput")
    out_d = nc.dram_tensor("out", [C, NQ], f32, kind="ExternalOutput")

    xf_r = xf[:, :].rearrange("(cb p) n -> p cb n", p=P)
    xs_r = xs[:, :].rearrange("(cb p) n -> p cb n", p=P)
    out_r = out_d[:, :].rearrange("(db p) n -> p db n", p=P)

    with tile.TileContext(nc) as tc:
        with (
            tc.tile_pool(name="persist", bufs=1) as persist,
            tc.tile_pool(name="att", bufs=2) as attp,
            tc.tile_pool(name="outp", bufs=2) as outp,
            tc.tile_pool(name="small", bufs=4) as small,
            tc.tile_pool(name="psa", bufs=4, space="PSUM") as psa,
            tc.tile_pool(name="psb", bufs=2, space="PSUM") as psb,
            tc.tile_pool(name="psc", bufs=2, space="PSUM") as psc,
        ):
            # ---- persistent SBUF tensors ----
            # float32r tiles feed matmuls; exact-f32 reads go through bitcast
            xs_sb = persist.tile([P, CB, NQ], f32r)     # 16 KB/part
            k_sb = persist.tile([P, CB, N], f32r)       # 32 KB/part
            q_sb = persist.tile([P, CB, NQ], f32r)      # 16 KB/part
            vT_sb = persist.tile([P, N // P, 260], f32r)  # 33.3 KB/part
            W_sb = {
                nm: persist.tile([P, CB, C], f32r, name=f"W_{nm}", tag=f"W_{nm}")
                for nm in Wd
            }
            b_sb = {
                nm: persist.tile([P, CB], f32, name=f"b_{nm}", tag=f"b_{nm}")
                for nm in bd
            }
            gamma_sb = persist.tile([P, CB], f32)
            beta_sb = persist.tile([P, CB], f32)
            ident_sb = persist.tile([P, P], f32)
            sel8_sb = persist.tile([P, GPB], f32)
            sel8T_sb = persist.tile([GPB, P], f32)
            scale_sb = persist.tile([P, CB], f32)    # per-channel gn scale
            tbias_sb = persist.tile([P, CB], f32r)   # per-channel gn shift
            bq_sb = persist.tile([P, CB], f32)       # folded q/k/v biases
            bk_sb = persist.tile([P, CB], f32)
            bv_sb = persist.tile([P, CB], f32)
            bo_sb = persist.tile([P, CB], f32)       # b_o + W_o^T b_v'
            stats_sb = persist.tile([P, CB, N // XCH, 6], f32)
            mv_sb = persist.tile([P, CB, 2], f32)
            me_sb = persist.tile([P, CB, 2], f32)
            eps_sb = persist.tile([GPB, 1], f32)
            nc.vector.memset(eps_sb, EPS)

            with tc.tile_pool(name="xp", bufs=1) as xp:
                x_sb = xp.tile([P, CB, N], f32r)    # 32 KB/part, scoped

                # ident first: warm-up matmuls depend only on it
                nc.sync.dma_start(out=ident_sb, in_=ident_d[:, :])

                # ---- load x (critical path); bn stats per chunk; PE
                # warm-ups tied to each chunk's stats keep HAM awake ----
                for ch in range(N // XCH):
                    sl = slice(ch * XCH, (ch + 1) * XCH)
                    eng = nc.sync if ch % 2 == 0 else nc.gpsimd
                    eng.dma_start(
                        out=x_sb[:, :, sl], in_=xf_r[:, :, sl].bitcast(f32r)
                    )
                    for cb in range(CB):
                        nc.vector.bn_stats(
                            out=stats_sb[:, cb, ch, :],
                            in_=x_sb[:, cb, sl].bitcast(f32),
                        )
                    ps_wu = psc.tile([P, 2], f32, tag="tr")
                    nc.tensor.matmul(
                        ps_wu,
                        lhsT=ident_sb,
                        rhs=stats_sb[:, 0, ch, 0:2],
                        start=True,
                        stop=True,
                    )

                # ---- remaining constant / weight / xs DMAs ----
                nc.sync.dma_start(out=sel8_sb, in_=sel8_d[:, :])
                nc.sync.dma_start(out=sel8T_sb, in_=sel8T_d[:, :])
                nc.sync.dma_start(
                    out=gamma_sb, in_=gamma_d[:].rearrange("(cb p) -> p cb", p=P)
                )
                nc.sync.dma_start(
                    out=beta_sb, in_=beta_d[:].rearrange("(cb p) -> p cb", p=P)
                )
                for nm in Wd:
                    nc.sync.dma_start(
                        out=W_sb[nm],
                        in_=Wd[nm][:, :]
                        .rearrange("(cb p) d -> p cb d", p=P)
                        .bitcast(f32r),
                    )
                    nc.sync.dma_start(
                        out=b_sb[nm], in_=bd[nm][:].rearrange("(cb p) -> p cb", p=P)
                    )
                for ch in range(NQ // MT):
                    sl = slice(ch * MT, (ch + 1) * MT)
                    nc.sync.dma_start(
                        out=xs_sb[:, :, sl], in_=xs_r[:, :, sl].bitcast(f32r)
                    )

                # ---- groupnorm scale/shift per channel ----
                for cb in range(CB):
                    nc.vector.bn_aggr(out=mv_sb[:, cb, :], in_=stats_sb[:, cb, :, :])
                    # me = (mean, E[x^2])
                    nc.vector.tensor_mul(
                        out=me_sb[:, cb, 1:2],
                        in0=mv_sb[:, cb, 0:1],
                        in1=mv_sb[:, cb, 0:1],
                    )
                    nc.vector.tensor_add(
                        out=me_sb[:, cb, 1:2],
                        in0=me_sb[:, cb, 1:2],
                        in1=mv_sb[:, cb, 1:2],
                    )
                    nc.vector.tensor_copy(
                        out=me_sb[:, cb, 0:1], in_=mv_sb[:, cb, 0:1]
                    )

                    # group-average across the 8 channels of each group
                    ps_g = psc.tile([GPB, 2], f32, tag="tr")
                    nc.tensor.matmul(
                        ps_g, lhsT=sel8_sb, rhs=me_sb[:, cb, :], start=True, stop=True
                    )
                    g2 = small.tile([GPB, 2], f32, tag="g2")
                    nc.vector.tensor_copy(out=g2, in_=ps_g)
                    gv = small.tile([GPB, 1], f32, tag="gv")
                    # gv = rstd = rsqrt(E[x^2] - mean^2 + eps)
                    nc.vector.tensor_mul(out=gv, in0=g2[:, 0:1], in1=g2[:, 0:1])
                    nc.vector.tensor_tensor(gv, g2[:, 1:2], gv, Alu.subtract)
                    nc.scalar.activation(out=gv, in_=gv, func=Act.Sqrt, bias=eps_sb)
                    nc.vector.reciprocal(out=gv, in_=gv)
                    nc.vector.tensor_copy(out=g2[:, 1:2], in_=gv)

                    # broadcast group (mean, rstd) back to the 128 channels
                    ps_bc = psc.tile([P, 2], f32, tag="tr")
                    nc.tensor.matmul(
                        ps_bc, lhsT=sel8T_sb, rhs=g2, start=True, stop=True
                    )
                    # scale = gamma*rstd ; tbias = beta - mean*scale
                    t1 = small.tile([P, 1], f32, tag="t1")
                    nc.vector.tensor_mul(
                        out=scale_sb[:, cb : cb + 1],
                        in0=gamma_sb[:, cb : cb + 1],
                        in1=ps_bc[:, 1:2],
                    )
                    nc.vector.tensor_mul(
                        out=t1, in0=ps_bc[:, 0:1], in1=scale_sb[:, cb : cb + 1]
                    )
                    nc.vector.tensor_tensor(
                        tbias_sb[:, cb : cb + 1],
                        beta_sb[:, cb : cb + 1],
                        t1,
                        Alu.subtract,
                    )

                # ---- fold groupnorm into q/k/v weights and biases ----
                for nm, bf_sb in (("q", bq_sb), ("k", bk_sb), ("v", bv_sb)):
                    for db in range(CB):
                        dsl = slice(db * P, (db + 1) * P)
                        ps_bb = psc.tile([P, 1], f32, tag="tr")
                        for cb in range(CB):
                            nc.tensor.matmul(
                                ps_bb,
                                lhsT=W_sb[nm][:, cb, dsl].bitcast(f32),
                                rhs=tbias_sb[:, cb : cb + 1].bitcast(f32),
                                start=(cb == 0),
                                stop=(cb == CB - 1),
                            )
                        nc.vector.tensor_add(
                            out=bf_sb[:, db : db + 1],
                            in0=ps_bb,
                            in1=b_sb[nm][:, db : db + 1],
                        )
                    for cb in range(CB):
                        nc.vector.tensor_scalar_mul(
                            out=W_sb[nm][:, cb, :],
                            in0=W_sb[nm][:, cb, :].bitcast(f32),
                            scalar1=scale_sb[:, cb : cb + 1],
                        )

                # v's bias adds bv[c] to the attention output (softmax rows
                # sum to 1), so fold it into the out-NIN bias instead:
                # bo_eff = b_o + W_o^T bv
                for db in range(CB):
                    dsl = slice(db * P, (db + 1) * P)
                    ps_cv = psc.tile([P, 1], f32, tag="tr", name=f"ps_cv_{db}")
                    for cb in range(CB):
                        nc.tensor.matmul(
                            ps_cv,
                            lhsT=W_sb["o"][:, cb, dsl].bitcast(f32),
                            rhs=bv_sb[:, cb : cb + 1],
                            start=(cb == 0),
                            stop=(cb == CB - 1),
                        )
                    nc.vector.tensor_add(
                        out=bo_sb[:, db : db + 1],
                        in0=ps_cv,
                        in1=b_sb["o"][:, db : db + 1],
                    )

                # ---- q / k / v NIN matmuls ----
                # ones columns of vT (softmax denominator trick)
                nc.vector.memset(vT_sb[:, :, 256:258].bitcast(f32), 1.0)

                def nin_tile(wname, bias_sb, src_sb, mt, dst_sb):
                    lsl = slice(mt * MT, (mt + 1) * MT)
                    for db in range(CB):
                        dsl = slice(db * P, (db + 1) * P)
                        ps = psa.tile([P, MT], f32, tag="mm")
                        for cb in range(CB):
                            nc.tensor.matmul(
                                ps,
                                lhsT=W_sb[wname][:, cb, dsl],
                                rhs=src_sb[:, cb, lsl],
                                start=(cb == 0),
                                stop=(cb == CB - 1),
                            )
                        # copy + per-channel bias; alternate engines so ACT
                        # and DVE drain psum banks concurrently
                        if db == 0:
                            nc.scalar.activation(
                                out=dst_sb[:, db, lsl],
                                in_=ps,
                                func=Act.Identity,
                                bias=bias_sb[:, db : db + 1],
                            )
                        else:
                            nc.vector.tensor_scalar_add(
                                out=dst_sb[:, db, lsl],
                                in0=ps,
                                scalar1=bias_sb[:, db : db + 1],
                            )

                def v_tile(mb):
                    # vT[m, d] directly: x block is the stationary operand
                    ps = psa.tile([P, C], f32, tag="mm", name=f"ps_v_{mb}")
                    for cb in range(CB):
                        nc.tensor.matmul(
                            ps,
                            lhsT=x_sb[:, cb, mb * P : (mb + 1) * P],
                            rhs=W_sb["v"][:, cb, :],
                            start=(cb == 0),
                            stop=(cb == CB - 1),
                        )
                    nc.vector.tensor_copy(out=vT_sb[:, mb, 0:C], in_=ps)

                # interleave k (ACT copies) and v (DVE copies) per m-tile so
                # both copy engines run concurrently; q (ACT) afterwards
                for mt in range(N // MT):
                    nin_tile("k", bk_sb, x_sb, mt, dst_sb=k_sb)
                    for j in range(MT // P):
                        v_tile(mt * (MT // P) + j)
                for mt in range(NQ // MT):
                    nin_tile("q", bq_sb, xs_sb, mt, dst_sb=q_sb)

            # ---- attention + out-NIN + residual, tiled over n ----
            with tc.tile_pool(name="wt", bufs=68) as wtp:
                wts_by_nt = {}

                def pass1(nt):
                    nsl = slice(nt * NT, (nt + 1) * NT)
                    wts = []
                    for mb in range(N // P):
                        ps_w = psa.tile([P, NT], f32, tag="mm")
                        for cb in range(CB):
                            nc.tensor.matmul(
                                ps_w,
                                lhsT=k_sb[:, cb, mb * P : (mb + 1) * P],
                                rhs=q_sb[:, cb, nsl],
                                start=(cb == 0),
                                stop=(cb == CB - 1),
                            )
                        wt = wtp.tile([P, NT], f32r, tag="wt")
                        nc.scalar.activation(
                            out=wt, in_=ps_w, func=Act.Exp, scale=SCALE
                        )
                        wts.append(wt)
                    wts_by_nt[nt] = wts

                def pass2(nt):
                    nsl = slice(nt * NT, (nt + 1) * NT)
                    wts = wts_by_nt.pop(nt)
                    attT = attp.tile([P, CB, NT], f32r, tag="attT")
                    for j in range(NT // P):
                        ps_o = psb.tile([P, 258], f32, tag="o", name=f"ps_o_{nt}_{j}")
                        for mb in range(N // P):
                            nc.tensor.matmul(
                                ps_o,
                                lhsT=wts[mb][:, j * P : (j + 1) * P],
                                rhs=vT_sb[:, mb, 0:258],
                                start=(mb == 0),
                                stop=(mb == N // P - 1),
                            )
                        rec = small.tile([P, 1], f32, tag="rec")
                        nc.vector.reciprocal(out=rec, in_=ps_o[:, 256:257])
                        att = attp.tile([P, C], f32, tag="att")
                        nc.vector.tensor_scalar_mul(
                            out=att, in0=ps_o[:, 0:C], scalar1=rec
                        )
                        for cb in range(CB):
                            ps_tr = psc.tile([P, P], f32, tag="tr", name=f"tr_{nt}_{j}_{cb}")
                            nc.tensor.transpose(
                                ps_tr, att[:, cb * P : (cb + 1) * P], ident_sb
                            )
                            nc.vector.tensor_copy(
                                out=attT[:, cb, j * P : (j + 1) * P], in_=ps_tr
                            )
                    for db in range(CB):
                        dsl = slice(db * P, (db + 1) * P)
                        ps_y = psa.tile([P, NT], f32, tag="mm")
                        for cb in range(CB):
                            nc.tensor.matmul(
                                ps_y,
                                lhsT=W_sb["o"][:, cb, dsl],
                                rhs=attT[:, cb, :],
                                start=(cb == 0),
                                stop=(cb == CB - 1),
                            )
                        o_sb = outp.tile([P, NT], f32, tag="o")
                        nc.scalar.activation(
                            out=o_sb,
                            in_=ps_y,
                            func=Act.Identity,
                            bias=bo_sb[:, db : db + 1],
                        )
                        nc.vector.tensor_add(
                            out=o_sb, in0=o_sb, in1=xs_sb[:, db, nsl].bitcast(f32)
                        )
                        nc.sync.dma_start(out=out_r[:, db, nsl], in_=o_sb)

                # software pipeline: pass1 runs one tile ahead so the exp
                # stream of tile nt hides behind pass1 matmuls of nt+1
                pass1(0)
                for nt in range(NQ // NT):
                    if nt + 1 < NQ // NT:
                        pass1(nt + 1)
                    pass2(nt)

    nc.compile()
    return nc


def _consts():
    ident = np.eye(P, dtype=np.float32)
    sel8 = np.zeros((P, GPB), np.float32)
    for p in range(P):
        sel8[p, p // (C // G)] = 1.0 / (C // G)
    sel8T = np.zeros((GPB, P), np.float32)
    for p in range(P):
        sel8T[p // (C // G), p] = 1.0
    return ident, sel8, sel8T


def kernel(x, gn_gamma, gn_beta, W0, b0, W1, b1, W2, b2, W3, b3):
    global _prog
    from concourse.bass_utils import run_bass_kernel_spmd

    if _prog is None:
        _prog = _build_program()

    ident, sel8, sel8T = _consts()
    f = lambda a: np.ascontiguousarray(np.asarray(a, dtype=np.float32))
    in_maps = []
    for j in range(NCORES):
        b, s = divmod(j, SPLIT)
        xb = f(np.asarray(x)[b].reshape(C, N))
        in_maps.append(
            {
                "xf": xb,
                "xs": f(xb[:, s * NQ : (s + 1) * NQ]),
                "Wq": f(W0), "bq": f(b0),
                "Wk": f(W1), "bk": f(b1),
                "Wv": f(W2), "bv": f(b2),
                "Wo": f(W3), "bo": f(b3),
                "gamma": f(gn_gamma), "beta": f(gn_beta),
                "ident": ident, "sel8": sel8, "sel8T": sel8T,
            }
        )
    try:
        res = run_bass_kernel_spmd(_prog, in_maps, list(range(NCORES)))
    except Exception:
        # transient device wedge (NRT_EXEC_UNIT_UNRECOVERABLE) — retry once
        res = run_bass_kernel_spmd(_prog, in_maps, list(range(NCORES)))
    out = np.empty((B, C, N), np.float32)
    for j in range(NCORES):
        b, s = divmod(j, SPLIT)
        out[b, :, s * NQ : (s + 1) * NQ] = res.results[j]["out"]
    return out.reshape(B, C, H, W)

